# revision 14
# baseline (speedup 1.0000x reference)
"""Gaussian L1-distance attention kernel for Trainium2 (8 NeuronCores).

Computes y[b,s,i,j] = exp(-(sum_d |x[b,i,d]-x[b,j,d]|)^2 / (2*sigma_s^2))
for x [4,2048,3] f32, sigmas [8] f32 -> y [4,8,2048,2048] f32 (512MB).

Strategy (v2, bf16):
- Upper-triangle packing: core c handles (batch c//2, sigma-half c%2)
  and computes the 16 row-tiles of the upper triangle (PACKW=17408
  packed columns); the host mirrors the lower triangle (symmetric).
- bf16 compute + bf16 output: halves the DMA-store floor vs f32
  (~50us/core instead of ~100us) and doubles/quadruples DVE throughput
  (2x_1p/4x_2p perf modes).  rel-err from bf16 is ~0.5%, well under
  the 2e-2 gate; host upcasts to f32.
- Engine split per chunk (tiles grouped into 7 column chunks so the
  elementwise work runs as few large instructions as possible):
  - DVE: per-tile x_d[cols]-x_d[row] subs (tensor_scalar, 4x bf16),
    one chunk-wide sign-bit AND (|.| on a u16 bitcast, 4x), s01 add
    and square (tensor_tensor, 2x).
  - GPSIMD: dist = s01 + t2 add (otherwise-idle engine).
  - ScalarE: 4 exps with per-partition scale APs -1/(2 sigma^2)
    computed on-chip from the sigmas input.  4 exp passes/core is the
    ScalarE floor (one exp per output element).
  - One DMA store per chunk covering all 4 sigma planes.
"""

import numpy as np

B, N, D, S = 4, 2048, 3, 8
NCORES = 8
NTILES = 16
S_LOC = 4                            # 4 sigma planes per core
# processing order small -> large -> small (short pipeline fill+drain)
PROC = [0, 2, 4, 6, 8, 10, 12, 14, 15, 13, 11, 9, 7, 5, 3, 1]
PW = [128 * (k + 1) for k in PROC]   # processed widths
PR = [15 - k for k in PROC]          # processed row-tiles
WOFF = [sum(PW[:j]) for j in range(NTILES)]
PACKW = sum(PW)                      # 17408
# compute chunks (consecutive PROC tiles): first/last tiny for fast
# ramp/drain; ACT+DMA run on groups of consecutive chunks (GROUPS) over a
# persistent SQ buffer, decoupling exp/store granularity from compute.
CHUNKS = [(0, 1), (1, 2), (2, 4), (4, 6), (6, 7), (7, 8), (8, 9), (9, 10),
          (10, 11), (11, 13), (13, 15), (15, 16)]
GROUPS = [(0, 1), (1, 3), (3, 4), (4, 6), (6, 8), (8, 10), (10, 12)]

_cached = None
TRACE_KW: dict = {}
LAST_RESULT = None


def _build():
    from concourse import mybir
    from concourse.bacc import Bacc
    from concourse.tile import TileContext

    f32 = mybir.dt.float32
    bf16 = mybir.dt.bfloat16
    u16 = mybir.dt.uint16
    Alu = mybir.AluOpType
    Act = mybir.ActivationFunctionType

    nc = Bacc()
    xr = nc.dram_tensor("xr", [128, D * N], bf16, kind="ExternalInput")
    xi = nc.dram_tensor("xi", [128, NTILES * D], f32, kind="ExternalInput")
    sg = nc.dram_tensor("sg", [128, S_LOC], f32, kind="ExternalInput")
    y = nc.dram_tensor("y", [S_LOC, 128, PACKW], bf16, kind="ExternalOutput")

    with TileContext(nc) as tc:
        with (
            tc.tile_pool(name="const", bufs=1) as cpool,
            tc.tile_pool(name="absd", bufs=3) as apool,
            tc.tile_pool(name="mid", bufs=3) as mpool,
            tc.tile_pool(name="dq", bufs=4) as dpool,
            tc.tile_pool(name="outp", bufs=2) as opool,
        ):
            xis = cpool.tile([128, NTILES * D], f32)
            nc.sync.dma_start(out=xis[:], in_=xi[:])
            sgs = cpool.tile([128, S_LOC], f32)
            nc.sync.dma_start(out=sgs[:], in_=sg[:])
            # x columns >= 1024 land first: the early (small) chunks are
            # high-row tiles that only read those columns.
            xrs = cpool.tile([128, D * N], bf16)
            for dd in range(D):
                nc.sync.dma_start(
                    out=xrs[:, dd * N + 1024:(dd + 1) * N],
                    in_=xr[:, dd * N + 1024:(dd + 1) * N])
            for dd in range(D):
                nc.sync.dma_start(
                    out=xrs[:, dd * N:dd * N + 1024],
                    in_=xr[:, dd * N:dd * N + 1024])

            # scale[:, i] = -1/(2*sigma_i^2)
            s2 = cpool.tile([128, S_LOC], f32)
            nc.vector.tensor_tensor(out=s2[:], in0=sgs[:], in1=sgs[:], op=Alu.mult)
            s2n = cpool.tile([128, S_LOC], f32)
            nc.vector.tensor_scalar_mul(s2n[:], s2[:], -2.0)
            rsc = cpool.tile([128, S_LOC], f32)
            nc.vector.reciprocal(out=rsc[:], in_=s2n[:])

            # persistent squared-distance buffer; chunks write disjoint
            # column ranges so there are no WAR hazards on it.
            sqb = cpool.tile([128, PACKW], bf16)

            for j0, j1 in CHUNKS:
                cw = sum(PW[j0:j1])
                t = apool.tile([128, 3 * cw], bf16, tag="t")
                # per-tile x_d[cols] - x_d[row_p] (tensor_scalar sub, 4x bf16)
                off = 0
                for j in range(j0, j1):
                    w, r = PW[j], PR[j]
                    c0 = r * 128
                    for d in range(D):
                        nc.vector.tensor_scalar(
                            t[:, d * cw + off:d * cw + off + w],
                            xrs[:, d * N + c0:d * N + c0 + w],
                            xis[:, j * D + d:j * D + d + 1],
                            None, Alu.subtract,
                        )
                    off += w
                # |.| via sign-bit clear on the whole chunk (one 4x pass)
                tu = t[:].bitcast(u16)
                nc.vector.tensor_scalar(tu, tu, 0x7FFF, None, Alu.bitwise_and)
                s01 = mpool.tile([128, cw], bf16, tag="s01")
                nc.vector.tensor_tensor(
                    out=s01[:], in0=t[:, 0:cw], in1=t[:, cw:2 * cw], op=Alu.add)
                # everything stays on Vector: GPSIMD's SBUF traffic was
                # measured to slow Vector by ~25% (worse than the work it
                # absorbs), so the Q7 engine is left idle on purpose.
                dist = dpool.tile([128, cw], bf16, tag="dist")
                nc.vector.tensor_tensor(
                    out=dist[:], in0=s01[:], in1=t[:, 2 * cw:3 * cw], op=Alu.add)
                nc.vector.tensor_tensor(
                    out=sqb[:, WOFF[j0]:WOFF[j0] + cw], in0=dist[:], in1=dist[:],
                    op=Alu.mult)

            for g0, g1 in GROUPS:
                gc0 = WOFF[CHUNKS[g0][0]]
                gw = sum(PW[CHUNKS[g0][0]:CHUNKS[g1 - 1][1]])
                o = opool.tile([128, S_LOC * gw], bf16, tag="o")
                for sl in range(S_LOC):
                    nc.scalar.activation(
                        out=o[:, sl * gw:(sl + 1) * gw],
                        in_=sqb[:, gc0:gc0 + gw], func=Act.Exp,
                        scale=rsc[:, sl:sl + 1],
                    )
                nc.sync.dma_start(
                    out=y[:, :, gc0:gc0 + gw].rearrange("s p w -> p s w"),
                    in_=o[:].rearrange("p (s w) -> p s w", s=S_LOC),
                )
    nc.finalize()
    return nc


def _pack_core_inputs(xb: np.ndarray, sig4: np.ndarray) -> dict:
    """xb: [N, D] batch slice; sig4: this core's 4 sigma values."""
    import ml_dtypes

    xrow = np.ascontiguousarray(
        np.broadcast_to(xb.T.reshape(1, D * N), (128, D * N))
    ).astype(ml_dtypes.bfloat16)
    xi = np.empty((128, NTILES * D), dtype=np.float32)
    for j, r in enumerate(PR):
        xi[:, j * D:(j + 1) * D] = xb[r * 128:(r + 1) * 128, :]
    sg = np.ascontiguousarray(
        np.broadcast_to(np.asarray(sig4, dtype=np.float32)[None, :], (128, S_LOC))
    )
    return {"xr": xrow, "xi": xi, "sg": sg}


def kernel(x: np.ndarray, sigmas: np.ndarray) -> np.ndarray:
    global _cached, LAST_RESULT
    from concourse import bass_utils

    x = np.ascontiguousarray(np.asarray(x, dtype=np.float32))
    sigmas = np.ascontiguousarray(np.asarray(sigmas, dtype=np.float32))

    if _cached is None:
        _cached = _build()
    nc = _cached

    in_maps = []
    for c in range(NCORES):
        b, h = c // 2, c % 2
        in_maps.append(_pack_core_inputs(x[b], sigmas[h * S_LOC:(h + 1) * S_LOC]))

    res = bass_utils.run_bass_kernel_spmd(
        nc, in_maps, core_ids=list(range(NCORES)), **TRACE_KW
    )
    LAST_RESULT = res

    out = np.empty((B, S, N, N), dtype=np.float32)
    for c in range(NCORES):
        b, h = c // 2, c % 2
        yl = res.results[c]["y"].astype(np.float32)    # [S_LOC, 128, PACKW]
        for j in range(NTILES):
            r, w = PR[j], PW[j]
            out[b, h * S_LOC:(h + 1) * S_LOC, r * 128:(r + 1) * 128, r * 128:] = (
                yl[:, :, WOFF[j]:WOFF[j] + w]
            )
    # mirror the lower triangle (symmetric)
    for r in range(NTILES - 1):
        src = out[:, :, r * 128:(r + 1) * 128, (r + 1) * 128:]
        out[:, :, (r + 1) * 128:, r * 128:(r + 1) * 128] = src.swapaxes(-1, -2)
    return out


# revision 17
# speedup vs baseline: 1.0154x; 1.0154x over previous
"""Gaussian L1-distance attention kernel for Trainium2 (8 NeuronCores).

Computes y[b,s,i,j] = exp(-(sum_d |x[b,i,d]-x[b,j,d]|)^2 / (2*sigma_s^2))
for x [4,2048,3] f32, sigmas [8] f32 -> y [4,8,2048,2048] f32 (512MB).

Strategy (v2, bf16):
- Upper-triangle packing: core c handles (batch c//2, sigma-half c%2)
  and computes the 16 row-tiles of the upper triangle (PACKW=17408
  packed columns); the host mirrors the lower triangle (symmetric).
- bf16 compute + bf16 output: halves the DMA-store floor vs f32
  (~50us/core instead of ~100us) and doubles/quadruples DVE throughput
  (2x_1p/4x_2p perf modes).  rel-err from bf16 is ~0.5%, well under
  the 2e-2 gate; host upcasts to f32.
- Engine split per chunk (tiles grouped into 7 column chunks so the
  elementwise work runs as few large instructions as possible):
  - DVE: per-tile x_d[cols]-x_d[row] subs (tensor_scalar, 4x bf16),
    one chunk-wide sign-bit AND (|.| on a u16 bitcast, 4x), s01 add
    and square (tensor_tensor, 2x).
  - GPSIMD: dist = s01 + t2 add (otherwise-idle engine).
  - ScalarE: 4 exps with per-partition scale APs -1/(2 sigma^2)
    computed on-chip from the sigmas input.  4 exp passes/core is the
    ScalarE floor (one exp per output element).
  - One DMA store per chunk covering all 4 sigma planes.
"""

import numpy as np

B, N, D, S = 4, 2048, 3, 8
NCORES = 8
NTILES = 16
S_LOC = 4                            # 4 sigma planes per core
# processing order small -> large -> small (short pipeline fill+drain)
PROC = [0, 2, 4, 6, 8, 10, 12, 14, 15, 13, 11, 9, 7, 5, 3, 1]
PW = [128 * (k + 1) for k in PROC]   # processed widths
PR = [15 - k for k in PROC]          # processed row-tiles
WOFF = [sum(PW[:j]) for j in range(NTILES)]
PACKW = sum(PW)                      # 17408
# compute chunks (consecutive PROC tiles): first/last tiny for fast
# ramp/drain; ACT+DMA run on groups of consecutive chunks (GROUPS) over a
# persistent SQ buffer, decoupling exp/store granularity from compute.
CHUNKS = [(0, 1), (1, 2), (2, 4), (4, 6), (6, 8), (8, 10), (10, 12),
          (12, 14), (14, 16)]
GROUPS = [(0, 3), (3, 4), (4, 5), (5, 6), (6, 7), (7, 8), (8, 9)]

_cached = None
TRACE_KW: dict = {}
LAST_RESULT = None


def _build():
    from concourse import mybir
    from concourse.bacc import Bacc
    from concourse.tile import TileContext

    f32 = mybir.dt.float32
    bf16 = mybir.dt.bfloat16
    u16 = mybir.dt.uint16
    Alu = mybir.AluOpType
    Act = mybir.ActivationFunctionType

    nc = Bacc()
    xr = nc.dram_tensor("xr", [128, D * N], bf16, kind="ExternalInput")
    xi = nc.dram_tensor("xi", [128, NTILES * D], f32, kind="ExternalInput")
    sg = nc.dram_tensor("sg", [128, S_LOC], f32, kind="ExternalInput")
    y = nc.dram_tensor("y", [S_LOC, 128, PACKW], bf16, kind="ExternalOutput")

    with TileContext(nc) as tc:
        with (
            tc.tile_pool(name="const", bufs=1) as cpool,
            tc.tile_pool(name="absd", bufs=2) as apool,
            tc.tile_pool(name="mid", bufs=2) as mpool,
            tc.tile_pool(name="outp", bufs=2) as opool,
        ):
            xis = cpool.tile([128, NTILES * D], f32)
            nc.sync.dma_start(out=xis[:], in_=xi[:])
            sgs = cpool.tile([128, S_LOC], f32)
            nc.sync.dma_start(out=sgs[:], in_=sg[:])
            # x columns >= 1024 land first: the early (small) chunks are
            # high-row tiles that only read those columns.
            xrs = cpool.tile([128, D * N], bf16)
            for dd in range(D):
                nc.sync.dma_start(
                    out=xrs[:, dd * N + 1024:(dd + 1) * N],
                    in_=xr[:, dd * N + 1024:(dd + 1) * N])
            for dd in range(D):
                nc.sync.dma_start(
                    out=xrs[:, dd * N:dd * N + 1024],
                    in_=xr[:, dd * N:dd * N + 1024])

            # scale[:, i] = -1/(2*sigma_i^2)
            s2 = cpool.tile([128, S_LOC], f32)
            nc.vector.tensor_tensor(out=s2[:], in0=sgs[:], in1=sgs[:], op=Alu.mult)
            s2n = cpool.tile([128, S_LOC], f32)
            nc.vector.tensor_scalar_mul(s2n[:], s2[:], -2.0)
            rsc = cpool.tile([128, S_LOC], f32)
            nc.vector.reciprocal(out=rsc[:], in_=s2n[:])

            # persistent squared-distance buffer; chunks write disjoint
            # column ranges so there are no WAR hazards on it.
            sqb = cpool.tile([128, PACKW], bf16)

            for j0, j1 in CHUNKS:
                cw = sum(PW[j0:j1])
                t = apool.tile([128, 3 * cw], bf16, tag="t")
                # per-tile x_d[cols] - x_d[row_p] (tensor_scalar sub, 4x bf16)
                off = 0
                for j in range(j0, j1):
                    w, r = PW[j], PR[j]
                    c0 = r * 128
                    for d in range(D):
                        nc.vector.tensor_scalar(
                            t[:, d * cw + off:d * cw + off + w],
                            xrs[:, d * N + c0:d * N + c0 + w],
                            xis[:, j * D + d:j * D + d + 1],
                            None, Alu.subtract,
                        )
                    off += w
                # |.| via sign-bit clear on the whole chunk (one 4x pass)
                tu = t[:].bitcast(u16)
                nc.vector.tensor_scalar(tu, tu, 0x7FFF, None, Alu.bitwise_and)
                s01 = mpool.tile([128, cw], bf16, tag="s01")
                nc.vector.tensor_tensor(
                    out=s01[:], in0=t[:, 0:cw], in1=t[:, cw:2 * cw], op=Alu.add)
                # everything stays on Vector: GPSIMD's SBUF traffic was
                # measured to slow Vector by ~25% (worse than the work it
                # absorbs), so the Q7 engine is left idle on purpose.
                dist = mpool.tile([128, cw], bf16, tag="dist")
                nc.vector.tensor_tensor(
                    out=dist[:], in0=s01[:], in1=t[:, 2 * cw:3 * cw], op=Alu.add)
                nc.vector.tensor_tensor(
                    out=sqb[:, WOFF[j0]:WOFF[j0] + cw], in0=dist[:], in1=dist[:],
                    op=Alu.mult)

            for g0, g1 in GROUPS:
                gc0 = WOFF[CHUNKS[g0][0]]
                gw = sum(PW[CHUNKS[g0][0]:CHUNKS[g1 - 1][1]])
                o = opool.tile([128, S_LOC * gw], bf16, tag="o")
                for sl in range(S_LOC):
                    nc.scalar.activation(
                        out=o[:, sl * gw:(sl + 1) * gw],
                        in_=sqb[:, gc0:gc0 + gw], func=Act.Exp,
                        scale=rsc[:, sl:sl + 1],
                    )
                nc.sync.dma_start(
                    out=y[:, :, gc0:gc0 + gw].rearrange("s p w -> p s w"),
                    in_=o[:].rearrange("p (s w) -> p s w", s=S_LOC),
                )
    nc.finalize()
    return nc


def _pack_core_inputs(xb: np.ndarray, sig4: np.ndarray) -> dict:
    """xb: [N, D] batch slice; sig4: this core's 4 sigma values."""
    import ml_dtypes

    xrow = np.ascontiguousarray(
        np.broadcast_to(xb.T.reshape(1, D * N), (128, D * N))
    ).astype(ml_dtypes.bfloat16)
    xi = np.empty((128, NTILES * D), dtype=np.float32)
    for j, r in enumerate(PR):
        xi[:, j * D:(j + 1) * D] = xb[r * 128:(r + 1) * 128, :]
    sg = np.ascontiguousarray(
        np.broadcast_to(np.asarray(sig4, dtype=np.float32)[None, :], (128, S_LOC))
    )
    return {"xr": xrow, "xi": xi, "sg": sg}


def kernel(x: np.ndarray, sigmas: np.ndarray) -> np.ndarray:
    global _cached, LAST_RESULT
    from concourse import bass_utils

    x = np.ascontiguousarray(np.asarray(x, dtype=np.float32))
    sigmas = np.ascontiguousarray(np.asarray(sigmas, dtype=np.float32))

    if _cached is None:
        _cached = _build()
    nc = _cached

    in_maps = []
    for c in range(NCORES):
        b, h = c // 2, c % 2
        in_maps.append(_pack_core_inputs(x[b], sigmas[h * S_LOC:(h + 1) * S_LOC]))

    res = bass_utils.run_bass_kernel_spmd(
        nc, in_maps, core_ids=list(range(NCORES)), **TRACE_KW
    )
    LAST_RESULT = res

    out = np.empty((B, S, N, N), dtype=np.float32)
    for c in range(NCORES):
        b, h = c // 2, c % 2
        yl = res.results[c]["y"].astype(np.float32)    # [S_LOC, 128, PACKW]
        for j in range(NTILES):
            r, w = PR[j], PW[j]
            out[b, h * S_LOC:(h + 1) * S_LOC, r * 128:(r + 1) * 128, r * 128:] = (
                yl[:, :, WOFF[j]:WOFF[j] + w]
            )
    # mirror the lower triangle (symmetric)
    for r in range(NTILES - 1):
        src = out[:, :, r * 128:(r + 1) * 128, (r + 1) * 128:]
        out[:, :, (r + 1) * 128:, r * 128:(r + 1) * 128] = src.swapaxes(-1, -2)
    return out


# revision 21
# speedup vs baseline: 1.0159x; 1.0005x over previous
"""Gaussian L1-distance attention kernel for Trainium2 (8 NeuronCores).

Computes y[b,s,i,j] = exp(-(sum_d |x[b,i,d]-x[b,j,d]|)^2 / (2*sigma_s^2))
for x [4,2048,3] f32, sigmas [8] f32 -> y [4,8,2048,2048] f32 (512MB).

Strategy (bf16, all-Vector dist pipeline):
- Upper-triangle packing: core c handles (batch c//2, sigma-half c%2)
  and computes the 16 row-tiles of the upper triangle (PACKW=17408
  packed columns); the host mirrors the lower triangle (symmetric).
- bf16 compute + bf16 output: halves the DMA-store floor vs f32
  (~50us/core instead of ~100us) and engages the DVE 2x_1p/4x_2p perf
  modes.  rel-err ~4e-3, well under the 2e-2 gate; host upcasts to f32.
- DVE does the whole dist pipeline: per-tile x_d[cols]-x_d[row] subs
  (tensor_scalar, 4x bf16), one chunk-wide sign-bit AND (|.| on a u16
  bitcast, 4x), the two adds and the square (tensor_tensor, 2x).
  GPSIMD is deliberately idle: measured on HW, its SBUF traffic slows
  DVE by ~25%, more than the work it absorbs.
- ScalarE: 4 exps with per-partition scale APs -1/(2 sigma^2) computed
  on-chip from the sigmas input.  4 exp passes/core is the ScalarE
  floor (one exp per stored element); ScalarE and DVE both sit at
  ~70-80us busy, DMA at ~50us.
- Squares land in one persistent packed SQ buffer (disjoint column
  ranges -> no WAR); exp+store run on coarser column groups over it.
"""

import numpy as np

B, N, D, S = 4, 2048, 3, 8
NCORES = 8
NTILES = 16
S_LOC = 4                            # 4 sigma planes per core
# processing order small -> large -> small (short pipeline fill+drain)
PROC = [0, 2, 4, 6, 8, 10, 12, 14, 15, 13, 11, 9, 7, 5, 3, 1]
PW = [128 * (k + 1) for k in PROC]   # processed widths
PR = [15 - k for k in PROC]          # processed row-tiles
WOFF = [sum(PW[:j]) for j in range(NTILES)]
PACKW = sum(PW)                      # 17408
# compute chunks (consecutive PROC tiles): first/last tiny for fast
# ramp/drain; ACT+DMA run on groups of consecutive chunks (GROUPS) over a
# persistent SQ buffer, decoupling exp/store granularity from compute.
CHUNKS = [(0, 1), (1, 2), (2, 4), (4, 6), (6, 8), (8, 10), (10, 12),
          (12, 14), (14, 16)]
GROUPS = [(0, 2), (2, 3), (3, 4), (4, 5), (5, 6), (6, 7), (7, 8), (8, 9)]

_cached = None
TRACE_KW: dict = {}
LAST_RESULT = None


def _build():
    from concourse import mybir
    from concourse.bacc import Bacc
    from concourse.tile import TileContext

    f32 = mybir.dt.float32
    bf16 = mybir.dt.bfloat16
    u16 = mybir.dt.uint16
    Alu = mybir.AluOpType
    Act = mybir.ActivationFunctionType

    nc = Bacc()
    xr = nc.dram_tensor("xr", [128, D * N], bf16, kind="ExternalInput")
    xi = nc.dram_tensor("xi", [128, NTILES * D], f32, kind="ExternalInput")
    sg = nc.dram_tensor("sg", [128, S_LOC], f32, kind="ExternalInput")
    y = nc.dram_tensor("y", [S_LOC, 128, PACKW], bf16, kind="ExternalOutput")

    with TileContext(nc) as tc:
        with (
            tc.tile_pool(name="const", bufs=1) as cpool,
            tc.tile_pool(name="work", bufs=2) as apool,
        ):
            xis = cpool.tile([128, NTILES * D], f32)
            nc.sync.dma_start(out=xis[:], in_=xi[:])
            sgs = cpool.tile([128, S_LOC], f32)
            nc.sync.dma_start(out=sgs[:], in_=sg[:])
            # x columns >= 1024 land first: the early (small) chunks are
            # high-row tiles that only read those columns.
            xrs = cpool.tile([128, D * N], bf16)
            for dd in range(D):
                nc.sync.dma_start(
                    out=xrs[:, dd * N + 1024:(dd + 1) * N],
                    in_=xr[:, dd * N + 1024:(dd + 1) * N])
            for dd in range(D):
                nc.sync.dma_start(
                    out=xrs[:, dd * N:dd * N + 1024],
                    in_=xr[:, dd * N:dd * N + 1024])

            # scale[:, i] = -1/(2*sigma_i^2)
            s2 = cpool.tile([128, S_LOC], f32)
            nc.vector.tensor_tensor(out=s2[:], in0=sgs[:], in1=sgs[:], op=Alu.mult)
            s2n = cpool.tile([128, S_LOC], f32)
            nc.vector.tensor_scalar_mul(s2n[:], s2[:], -2.0)
            rsc = cpool.tile([128, S_LOC], f32)
            nc.vector.reciprocal(out=rsc[:], in_=s2n[:])

            # persistent squared-distance buffer; chunks write disjoint
            # column ranges so there are no WAR hazards on it.
            sqb = cpool.tile([128, PACKW], bf16)

            for j0, j1 in CHUNKS:
                cw = sum(PW[j0:j1])
                t = apool.tile([128, 3 * cw], bf16, tag="t")
                # per-tile x_d[cols] - x_d[row_p] (tensor_scalar sub, 4x bf16)
                off = 0
                for j in range(j0, j1):
                    w, r = PW[j], PR[j]
                    c0 = r * 128
                    for d in range(D):
                        nc.vector.tensor_scalar(
                            t[:, d * cw + off:d * cw + off + w],
                            xrs[:, d * N + c0:d * N + c0 + w],
                            xis[:, j * D + d:j * D + d + 1],
                            None, Alu.subtract,
                        )
                    off += w
                # |.| via sign-bit clear on the whole chunk (one 4x pass)
                tu = t[:].bitcast(u16)
                nc.vector.tensor_scalar(tu, tu, 0x7FFF, None, Alu.bitwise_and)
                s01 = apool.tile([128, cw], bf16, tag="s01")
                nc.vector.tensor_tensor(
                    out=s01[:], in0=t[:, 0:cw], in1=t[:, cw:2 * cw], op=Alu.add)
                # everything stays on Vector: GPSIMD's SBUF traffic was
                # measured to slow Vector by ~25% (worse than the work it
                # absorbs), so the Q7 engine is left idle on purpose.
                dist = apool.tile([128, cw], bf16, tag="dist")
                nc.vector.tensor_tensor(
                    out=dist[:], in0=s01[:], in1=t[:, 2 * cw:3 * cw], op=Alu.add)
                nc.vector.tensor_tensor(
                    out=sqb[:, WOFF[j0]:WOFF[j0] + cw], in0=dist[:], in1=dist[:],
                    op=Alu.mult)

            for g0, g1 in GROUPS:
                gc0 = WOFF[CHUNKS[g0][0]]
                gw = sum(PW[CHUNKS[g0][0]:CHUNKS[g1 - 1][1]])
                o = apool.tile([128, S_LOC * gw], bf16, tag="o")
                for sl in range(S_LOC):
                    nc.scalar.activation(
                        out=o[:, sl * gw:(sl + 1) * gw],
                        in_=sqb[:, gc0:gc0 + gw], func=Act.Exp,
                        scale=rsc[:, sl:sl + 1],
                    )
                nc.sync.dma_start(
                    out=y[:, :, gc0:gc0 + gw].rearrange("s p w -> p s w"),
                    in_=o[:].rearrange("p (s w) -> p s w", s=S_LOC),
                )
    nc.finalize()
    return nc


def _pack_core_inputs(xb: np.ndarray, sig4: np.ndarray) -> dict:
    """xb: [N, D] batch slice; sig4: this core's 4 sigma values."""
    import ml_dtypes

    xrow = np.ascontiguousarray(
        np.broadcast_to(xb.T.reshape(1, D * N), (128, D * N))
    ).astype(ml_dtypes.bfloat16)
    xi = np.empty((128, NTILES * D), dtype=np.float32)
    for j, r in enumerate(PR):
        xi[:, j * D:(j + 1) * D] = xb[r * 128:(r + 1) * 128, :]
    sg = np.ascontiguousarray(
        np.broadcast_to(np.asarray(sig4, dtype=np.float32)[None, :], (128, S_LOC))
    )
    return {"xr": xrow, "xi": xi, "sg": sg}


def kernel(x: np.ndarray, sigmas: np.ndarray) -> np.ndarray:
    global _cached, LAST_RESULT
    from concourse import bass_utils

    x = np.ascontiguousarray(np.asarray(x, dtype=np.float32))
    sigmas = np.ascontiguousarray(np.asarray(sigmas, dtype=np.float32))

    if _cached is None:
        _cached = _build()
    nc = _cached

    in_maps = []
    for c in range(NCORES):
        b, h = c // 2, c % 2
        in_maps.append(_pack_core_inputs(x[b], sigmas[h * S_LOC:(h + 1) * S_LOC]))

    res = bass_utils.run_bass_kernel_spmd(
        nc, in_maps, core_ids=list(range(NCORES)), **TRACE_KW
    )
    LAST_RESULT = res

    out = np.empty((B, S, N, N), dtype=np.float32)
    for c in range(NCORES):
        b, h = c // 2, c % 2
        yl = res.results[c]["y"].astype(np.float32)    # [S_LOC, 128, PACKW]
        for j in range(NTILES):
            r, w = PR[j], PW[j]
            out[b, h * S_LOC:(h + 1) * S_LOC, r * 128:(r + 1) * 128, r * 128:] = (
                yl[:, :, WOFF[j]:WOFF[j] + w]
            )
    # mirror the lower triangle (symmetric)
    for r in range(NTILES - 1):
        src = out[:, :, r * 128:(r + 1) * 128, (r + 1) * 128:]
        out[:, :, (r + 1) * 128:, r * 128:(r + 1) * 128] = src.swapaxes(-1, -2)
    return out
